# revision 1
# baseline (speedup 1.0000x reference)
"""Per-sample Gaussian blur (bilateral-filter reference) on 8 Trainium2 cores.

Math: for each sample b, the reference does a depthwise conv with a separable
normalized Gaussian k x k kernel (k in {5..9} from params[b,0], sigma from
params[b,1]), pad=k//2, then for even k a bilinear resize (H+1,W+1)->(H,W).
Both the 1-D conv and the resize are linear maps along one axis, so the whole
per-sample op is  out_c = A @ X_c @ A^T  with a single banded (|i-j|<=4)
384x384 matrix A = (resize) @ Toeplitz(gauss1d) built on the host.

Device kernel (pure data parallel, one sample per core): for each channel,
two tensor-engine passes with A^T as the moving operand:
  pass1: P^T[w,i] = sum_h X[h,w] * AT[h,i]   (lhsT = X chunk,  rhs = AT)
  pass2: O[i,j]   = sum_w P^T[w,i] * AT[w,j] (lhsT = P^T chunk, rhs = AT)
No transposes needed anywhere. In banded mode each contraction-chunk matmul
covers only its chunk's nonzero A^T column band (416 of 1152 columns per
output tile), relying on probed PSUM has_written semantics: one start=True
matmul opens the bank group, later start=False matmuls overwrite untouched
columns and accumulate on the small band overlaps.
"""

import numpy as np

_H = 384
_C = 64
_NCORES = 8

# precision / strategy config (hardcoded at submission)
IN_16 = True    # ship x and A^T to the device in fp16 (halves input DMA)
OUT_16 = True   # device writes fp16, host upcasts (halves output DMA)
BANDED = True   # banded matmuls (needs 16-bit operands for 1 cyc/row at N<256)

_prog_cache = {}


def _sigmoid32(v):
    v = np.asarray(v, dtype=np.float32)
    return (1.0 / (1.0 + np.exp(-v.astype(np.float64)))).astype(np.float32)


def _gauss1d(k, sigma):
    c = np.arange(k, dtype=np.float64) - k // 2
    g = np.exp(-(c * c) / (2.0 * float(sigma) ** 2))
    return g / g.sum()


def _build_A(k, sigma, H=_H):
    """Combined conv(+resize for even k) operator along one axis (H x H)."""
    pad = k // 2
    Ho = H + 2 * pad - k + 1  # H odd k, H+1 even k
    g = _gauss1d(k, sigma)
    S = np.zeros((Ho, H), dtype=np.float64)
    for i in range(Ho):
        lo = max(0, i - pad)
        hi = min(H, i - pad + k)
        for m in range(lo, hi):
            S[i, m] = g[m - i + pad]
    if Ho == H:
        return S.astype(np.float32)
    # bilinear resize Ho -> H, half-pixel centers, no antialias
    R = np.zeros((H, Ho), dtype=np.float64)
    scale = Ho / H
    for i in range(H):
        src = (i + 0.5) * scale - 0.5
        i0 = int(np.floor(src))
        t = src - i0
        i0c = min(max(i0, 0), Ho - 1)
        i1c = min(max(i0 + 1, 0), Ho - 1)
        R[i, i0c] += 1.0 - t
        R[i, i1c] += t
    return (R @ S).astype(np.float32)


def _build_program():
    """One SPMD Bass program: x (64,384,384) + at (384,384) -> out."""
    if "nc" in _prog_cache:
        return _prog_cache["nc"]

    from contextlib import ExitStack
    import concourse.bacc as bacc
    import concourse.mybir as mybir
    import concourse.tile as tile

    f32 = mybir.dt.float32
    f32r = mybir.dt.float32r
    f16 = mybir.dt.float16

    dt_in = f16 if IN_16 else f32r
    dt_out = f16 if OUT_16 else f32
    banded = BANDED and IN_16

    nc = bacc.Bacc(None, target_bir_lowering=False)
    x_d = nc.declare_dram_parameter("x", [_C, _H, _H], dt_in, isOutput=False)
    at_d = nc.declare_dram_parameter("at", [_H, _H], dt_in, isOutput=False)
    out_d = nc.declare_dram_parameter("out", [_C, _H, _H], dt_out, isOutput=True)

    # Banded matmul plan per output tile: (chunk, col_lo, col_hi, start).
    # PSUM semantics (probed on HW): start=True resets has_written for the
    # whole bank (data intact); a write to an hw=0 element overwrites, to an
    # hw=1 element accumulates. A^T chunk kc only has nonzero columns in
    # [128*kc-4, 128*kc+131], so each matmul covers just its own band
    # (8-aligned), overwriting fresh columns and accumulating on the two
    # 8..16-column overlaps, which the issue order makes well-defined.
    if banded:
        MM_PLAN = [(0, 0, 136, True), (1, 120, 264, False), (2, 248, 384, False)]
    else:
        MM_PLAN = [(0, 0, 384, True), (1, 0, 384, False), (2, 0, 384, False)]

    with tile.TileContext(nc) as tc, ExitStack() as ctx:
        at_pool = ctx.enter_context(tc.tile_pool(name="at", bufs=1))
        x_pool = ctx.enter_context(tc.tile_pool(name="x", bufs=4))
        pt_pool = ctx.enter_context(tc.tile_pool(name="pt", bufs=2))
        o_pool = ctx.enter_context(tc.tile_pool(name="o", bufs=4))
        # pass1 psum: one merged 3-bank tile per channel (single DVE copy out);
        # pass2: single-bank tiles copied per-subtile on ACT. (The flipped
        # arrangement — singles for pass1, merged for pass2 — lowers total
        # copy work ~9us via fewer ACT overheads but serializes the pipeline
        # to 65% engine occupancy: sim 133.6us vs 106.2us. Keep this one.)
        ps1 = ctx.enter_context(tc.tile_pool(name="ps1", bufs=2, space="PSUM"))
        ps2 = ctx.enter_context(tc.tile_pool(name="ps2", bufs=2, space="PSUM"))

        # A^T resident in SBUF: at_t[p, kc, i] = AT[kc*128 + p, i]
        # (gpsimd queue so it loads in parallel with channel 0's x DMA on sync)
        at_t = at_pool.tile([128, 3, _H], dt_in)
        nc.gpsimd.dma_start(
            at_t[:], at_d[:].rearrange("(kc p) i -> p kc i", p=128)
        )

        # Software-pipelined emission, one channel of skew: produce pass1(c)
        # before consuming pass2(c-1). The scheduler follows emission-order
        # priorities, so this keeps the DVE merged-copy stream saturated
        # instead of stalling ~0.5us per channel behind pass2 matmuls.
        n_mm = len(MM_PLAN)
        pend = {}
        for step in range(_C + 1):
            if step < _C:
                c = step
                x_t = x_pool.tile([128, 3, _H], dt_in)
                xs = x_d[c].rearrange("(kk p) w -> p kk w", p=128)
                if c == 0:
                    # fill: per-chunk loads let channel 0's first matmul
                    # start as soon as chunk 0 lands
                    for kk in range(3):
                        nc.sync.dma_start(x_t[:, kk, :], xs[:, kk, :])
                else:
                    nc.sync.dma_start(x_t[:], xs)
                pt_t = pt_pool.tile([128, 3, _H], dt_in)
                p1 = ps1.tile([128, 3, 512], f32)
                for m in range(3):
                    for i_mm, (kc, lo, hi, st) in enumerate(MM_PLAN):
                        nc.tensor.matmul(
                            p1[:, m, lo:hi],
                            x_t[:, kc, 128 * m : 128 * (m + 1)],
                            at_t[:, kc, lo:hi],
                            start=st,
                            stop=(i_mm == n_mm - 1),
                            skip_group_check=True,
                        )
                # channels 0-1's copies go to ACT, which is otherwise idle
                # through the pipeline fill (c<=2 overshoots: DVE then idles
                # too long before its first merged copy)
                if c <= 1:
                    nc.scalar.copy(pt_t[:], p1[:, :, 0:_H])
                else:
                    nc.vector.tensor_copy(pt_t[:], p1[:, :, 0:_H])
                pend[c] = pt_t
            if step >= 1:
                c = step - 1
                pt_t = pend.pop(c)
                o_t = o_pool.tile([128, 3, _H], dt_out)
                for it in range(3):
                    p2 = ps2.tile([128, 512], f32)
                    for i_mm, (kc, lo, hi, st) in enumerate(MM_PLAN):
                        nc.tensor.matmul(
                            p2[:, lo:hi],
                            pt_t[:, kc, 128 * it : 128 * (it + 1)],
                            at_t[:, kc, lo:hi],
                            start=st,
                            stop=(i_mm == n_mm - 1),
                            skip_group_check=True,
                        )
                    # Divert the LAST pass2 copy of every 3rd channel to DVE:
                    # unlike mid-channel (it<2) diversions, an it==2 copy is
                    # not on ACT's next-work dependency chain (the next
                    # channel's first copy waits on DVE's merged pass1 copy,
                    # which the pipelined emission places ahead in DVE's
                    # queue), so this sheds ACT work without stalling it.
                    # Even 1-in-3 spacing is load-bearing: clustered
                    # diversions overload DVE locally and collapse the
                    # pipeline (measured +8us).
                    if (it == 2 and c % 3 == 2) or (c == _C - 1 and it == 2):
                        nc.vector.tensor_copy(o_t[:, it, :], p2[:, 0:_H])
                    else:
                        nc.scalar.copy(o_t[:, it, :], p2[:, 0:_H])
                nc.gpsimd.dma_start(
                    out_d[c].rearrange("(m p) j -> p m j", p=128), o_t[:]
                )

    nc.finalize()
    _prog_cache["nc"] = nc
    return nc


def kernel(x, params, _trace=False):
    from concourse.bass_utils import run_bass_kernel_spmd
    import concourse.mybir as mybir

    x = np.ascontiguousarray(np.asarray(x, dtype=np.float32))
    params = np.asarray(params, dtype=np.float32)
    B = x.shape[0]
    assert x.shape == (_NCORES, _C, _H, _H), x.shape

    k_int = np.trunc(params[:, 0].astype(np.float32))
    k_sel = np.floor(
        np.float32(5.0) + np.float32(5.0) * _sigmoid32(k_int)
    ).astype(np.int32)
    sigma = np.float32(0.5) + np.float32(4.5) * _sigmoid32(params[:, 1])

    np_in = mybir.dt.np(mybir.dt.float16 if IN_16 else mybir.dt.float32)

    nc = _build_program()
    in_maps = []
    for b in range(B):
        A = _build_A(int(k_sel[b]), float(sigma[b]))
        at = np.ascontiguousarray(A.T)
        in_maps.append(
            {"x": x[b].astype(np_in), "at": at.astype(np_in)}
        )

    res = run_bass_kernel_spmd(
        nc, in_maps, list(range(_NCORES)), trace=_trace
    )
    out = np.stack(
        [np.asarray(res.results[b]["out"]).astype(np.float32) for b in range(B)]
    )
    if _trace:
        return out, res
    return out



# revision 2
# speedup vs baseline: 1.0207x; 1.0207x over previous
"""Per-sample Gaussian blur on 8 Trainium2 cores — v3.

Math (as baseline): out_c = A @ X_c @ A^T via two banded tensor-engine
passes; A built on host per sample from (k, sigma).

v3 vs baseline:
- x ships as float8_e3m4 (1 byte), host-prepacked partition-major so DMA
  descriptors are 1152B (full 360GB/s). Mixed-dtype matmul: lhsT=e3
  stationary, rhs=A^T fp16 moving — same 1 cyc/row as fp16. Input DMA
  26us instead of 52us. Measured end-to-end rel_fro error 1.34e-2.
- PSUM repartition: all drains are 2-bank pair copies with >=1-channel
  reuse distance (kills the ps2 single-bank reuse stall of the 2-pool
  layout): ps1p[2]x2 (pass1 m0,m1), psPQ[2]x1 (pass1 m2(c) paired with
  pass2 it0(c-1)), psRS[2]x1 (pass2 it1+it2 of c-1). The PQ pair drains
  into a combined staging tile st(c) = [pt_m2(c) | O_it0..it2(c-1)] so
  one copy serves both passes; output DMA reads st[:, 1:4].
- Copy work balanced across ACT and DVE by assignment tables.
"""

import numpy as np

_H = 384
_C = 64
_NCORES = 8

BATCH_IN = 4         # channels per input DMA instruction

# Engine assignment for the three pair-copies per channel (tuned on the
# cost model; the clean 2-channel alternation schedules better than exact
# DVE/ACT load balance): m01 -> ACT; pq alternates; rs -> DVE.
PQ_ACT = {c for c in range(_C) if c % 2 == 1}
RS_ACT = set()
M01_DVE = set()                   # channels whose m01 pair-copy goes to DVE
BUFS_T1 = 3
BUFS_ST = 3
PS_BUFS = (1, 1, 2)               # bufs for (ps1p, psPQ, psRS); sum*2 <= 8

_prog_cache = {}


def _sigmoid32(v):
    v = np.asarray(v, dtype=np.float32)
    return (1.0 / (1.0 + np.exp(-v.astype(np.float64)))).astype(np.float32)


def _gauss1d(k, sigma):
    c = np.arange(k, dtype=np.float64) - k // 2
    g = np.exp(-(c * c) / (2.0 * float(sigma) ** 2))
    return g / g.sum()


def _build_A(k, sigma, H=_H):
    """Combined conv(+resize for even k) operator along one axis (H x H)."""
    pad = k // 2
    Ho = H + 2 * pad - k + 1
    g = _gauss1d(k, sigma)
    S = np.zeros((Ho, H), dtype=np.float64)
    for i in range(Ho):
        lo = max(0, i - pad)
        hi = min(H, i - pad + k)
        for m in range(lo, hi):
            S[i, m] = g[m - i + pad]
    if Ho == H:
        return S.astype(np.float32)
    R = np.zeros((H, Ho), dtype=np.float64)
    scale = Ho / H
    for i in range(H):
        src = (i + 0.5) * scale - 0.5
        i0 = int(np.floor(src))
        t = src - i0
        i0c = min(max(i0, 0), Ho - 1)
        i1c = min(max(i0 + 1, 0), Ho - 1)
        R[i, i0c] += 1.0 - t
        R[i, i1c] += t
    return (R @ S).astype(np.float32)


def _build_program():
    key = ("v3", BATCH_IN, tuple(sorted(PQ_ACT)), tuple(sorted(RS_ACT)),
           tuple(sorted(M01_DVE)), PS_BUFS, BUFS_T1, BUFS_ST)
    if key in _prog_cache:
        return _prog_cache[key]

    from contextlib import ExitStack
    import concourse.bacc as bacc
    import concourse.mybir as mybir
    import concourse.tile as tile

    f32 = mybir.dt.float32
    f16 = mybir.dt.float16
    e3 = mybir.dt.float8e3

    nc = bacc.Bacc(None, target_bir_lowering=False)
    # x prepacked on host: x8[c, p, kc*384 + w] = e3m4(X[c, kc*128 + p, w])
    x_d = nc.declare_dram_parameter("x8", [_C, 128, 3 * _H], e3, isOutput=False)
    at_d = nc.declare_dram_parameter("at", [_H, _H], f16, isOutput=False)
    out_d = nc.declare_dram_parameter("out", [_C, _H, _H], f16, isOutput=True)

    # Banded plan: A^T chunk kc has nonzero cols only in [128*kc-4, 128*kc+132)
    MM_PLAN = [(0, 0, 132, True), (1, 124, 260, False), (2, 252, 384, False)]
    n_mm = len(MM_PLAN)

    def cp(act, dst, src):
        if act:
            nc.scalar.copy(dst, src)
        else:
            nc.vector.tensor_copy(dst, src)

    with tile.TileContext(nc) as tc, ExitStack() as ctx:
        at_pool = ctx.enter_context(tc.tile_pool(name="at", bufs=1))
        x_pool = ctx.enter_context(tc.tile_pool(name="x", bufs=4))
        t1_pool = ctx.enter_context(tc.tile_pool(name="t1", bufs=BUFS_T1))
        st_pool = ctx.enter_context(tc.tile_pool(name="st", bufs=BUFS_ST))
        ps1p = ctx.enter_context(
            tc.tile_pool(name="ps1p", bufs=PS_BUFS[0], space="PSUM"))
        psPQ = ctx.enter_context(
            tc.tile_pool(name="psPQ", bufs=PS_BUFS[1], space="PSUM"))
        psRS = ctx.enter_context(
            tc.tile_pool(name="psRS", bufs=PS_BUFS[2], space="PSUM"))

        at_t = at_pool.tile([128, 3, _H], f16)
        nc.gpsimd.dma_start(
            at_t[:], at_d[:].rearrange("(kc p) i -> p kc i", p=128)
        )

        x_tiles = {}
        pend = {}   # c -> (t1_t, st_t) holding P^T of channel c
        for step in range(_C + 1):
            if step < _C and step % BATCH_IN == 0:
                bi = step // BATCH_IN
                xt = x_pool.tile([128, BATCH_IN, 3, _H], e3, name="xt")
                src = x_d[step : step + BATCH_IN].rearrange(
                    "c p (kc w) -> p c kc w", kc=3
                )
                if bi == 0:
                    for i in range(BATCH_IN):
                        nc.sync.dma_start(xt[:, i], src[:, i])
                else:
                    nc.sync.dma_start(xt[:], src)
                for i in range(BATCH_IN):
                    x_tiles[step + i] = (xt, i)

            c = step            # pass1 channel
            d = step - 1        # pass2 channel
            pq = psPQ.tile([128, 2, 512], f32, name="pq")
            rs = psRS.tile([128, 2, 512], f32, name="rs") if d >= 0 else None
            st_t = st_pool.tile([128, 4, _H], f16, name="stt")

            def mm(dst, lhsT, plan_kc, lo, hi, st_flag, stop_flag):
                nc.tensor.matmul(
                    dst, lhsT, at_t[:, plan_kc, lo:hi],
                    start=st_flag, stop=stop_flag, skip_group_check=True,
                )

            if c < _C:
                xt, xi = x_tiles.pop(c)
                t1_t = t1_pool.tile([128, 2, _H], f16, name="t1t")
                # pass1 m0, m1 -> ps1p pair
                p1p = ps1p.tile([128, 2, 512], f32, name="p1p")
                for m in range(2):
                    for i_mm, (kc, lo, hi, stf) in enumerate(MM_PLAN):
                        mm(p1p[:, m, lo:hi], xt[:, xi, kc, 128 * m : 128 * (m + 1)],
                           kc, lo, hi, stf, i_mm == n_mm - 1)
                # pair-copy m0,m1 — emitted before m2 group
                cp(c not in M01_DVE, t1_t[:], p1p[:, :, 0:_H])
                # pass1 m2 -> pq slot 0
                for i_mm, (kc, lo, hi, stf) in enumerate(MM_PLAN):
                    mm(pq[:, 0, lo:hi], xt[:, xi, kc, 256:384],
                       kc, lo, hi, stf, i_mm == n_mm - 1)
                pend[c] = (t1_t, st_t)

            if d >= 0:
                t1_p, st_p = pend.pop(d)

                def lhs2(kc, it):
                    sl = slice(128 * it, 128 * (it + 1))
                    if kc < 2:
                        return t1_p[:, kc, sl]
                    return st_p[:, 0, sl]

                # it0 -> pq slot 1
                for i_mm, (kc, lo, hi, stf) in enumerate(MM_PLAN):
                    mm(pq[:, 1, lo:hi], lhs2(kc, 0), kc, lo, hi, stf,
                       i_mm == n_mm - 1)
                # pair-copy [pt_m2(c) | O_it0(d)] -> st(c)[:, 0:2]
                if c < _C:
                    cp(c in PQ_ACT, st_t[:, 0:2, :], pq[:, :, 0:_H])
                else:
                    cp(True, st_t[:, 1:2, :], pq[:, 1:2, 0:_H])
                # it1, it2 -> rs
                for it in (1, 2):
                    for i_mm, (kc, lo, hi, stf) in enumerate(MM_PLAN):
                        mm(rs[:, it - 1, lo:hi], lhs2(kc, it), kc, lo, hi, stf,
                           i_mm == n_mm - 1)
                cp(d in RS_ACT, st_t[:, 2:4, :], rs[:, :, 0:_H])
                # output channel d lives in st(c)[:, 1:4]
                nc.gpsimd.dma_start(
                    out_d[d].rearrange("(m p) j -> p m j", p=128),
                    st_t[:, 1:4, :],
                )
            elif c < _C:
                # step 0: no pass2; drain only pt_m2 slot
                cp(True, st_t[:, 0:1, :], pq[:, 0:1, 0:_H])

    nc.finalize()
    _prog_cache[key] = nc
    return nc


def _pack_x(xb, np_e3):
    """xb (64,384,384) f32 -> (64,128,1152) e3m4: [c, p, kc*384+w]."""
    v = xb.reshape(_C, 3, 128, _H).transpose(0, 2, 1, 3).reshape(_C, 128, 3 * _H)
    return np.ascontiguousarray(v.astype(np_e3))


def kernel(x, params, _trace=False):
    from concourse.bass_utils import run_bass_kernel_spmd
    import concourse.mybir as mybir

    x = np.ascontiguousarray(np.asarray(x, dtype=np.float32))
    params = np.asarray(params, dtype=np.float32)
    B = x.shape[0]
    assert x.shape == (_NCORES, _C, _H, _H), x.shape

    k_int = np.trunc(params[:, 0].astype(np.float32))
    k_sel = np.floor(
        np.float32(5.0) + np.float32(5.0) * _sigmoid32(k_int)
    ).astype(np.int32)
    sigma = np.float32(0.5) + np.float32(4.5) * _sigmoid32(params[:, 1])

    np_e3 = mybir.dt.np(mybir.dt.float8e3)

    nc = _build_program()
    in_maps = []
    for b in range(B):
        A = _build_A(int(k_sel[b]), float(sigma[b]))
        at = np.ascontiguousarray(A.T.astype(np.float16))
        in_maps.append({"x8": _pack_x(x[b], np_e3), "at": at})

    res = run_bass_kernel_spmd(nc, in_maps, list(range(_NCORES)), trace=_trace)
    out = np.stack(
        [np.asarray(res.results[b]["out"]).astype(np.float32) for b in range(B)]
    )
    if _trace:
        return out, res
    return out


# revision 3
# speedup vs baseline: 1.0824x; 1.0604x over previous
"""Per-sample Gaussian blur on 8 Trainium2 cores — v3.

Math (as baseline): out_c = A @ X_c @ A^T via two banded tensor-engine
passes; A built on host per sample from (k, sigma).

v3 vs baseline:
- x ships as float8_e3m4 (1 byte), host-prepacked partition-major so DMA
  descriptors are 1152B (full 360GB/s). Mixed-dtype matmul: lhsT=e3
  stationary, rhs=A^T fp16 moving — same 1 cyc/row as fp16. Input DMA
  26us instead of 52us. Measured end-to-end rel_fro error 1.34e-2.
- PSUM repartition: all drains are 2-bank pair copies with >=1-channel
  reuse distance (kills the ps2 single-bank reuse stall of the 2-pool
  layout): ps1p[2]x2 (pass1 m0,m1), psPQ[2]x1 (pass1 m2(c) paired with
  pass2 it0(c-1)), psRS[2]x1 (pass2 it1+it2 of c-1). The PQ pair drains
  into a combined staging tile st(c) = [pt_m2(c) | O_it0..it2(c-1)] so
  one copy serves both passes; output DMA reads st[:, 1:4].
- Copy work balanced across ACT and DVE by assignment tables.
"""

import numpy as np

_H = 384
_C = 64
_NCORES = 8

BATCH_IN = 4         # channels per input DMA instruction

# Engine assignment for the three pair-copies per channel, tuned on the
# cost model: odd channels run m01 on DVE and pq+rs on ACT; even channels
# the mirror image. Alternating the m01 engine decouples the tight
# single-buffered ps1p mm->copy->mm cycle from any one engine's in-order
# queue (96.7us vs 102.6us for fixed assignment).
PQ_ACT = {c for c in range(_C) if c % 2 == 1}
RS_ACT = {c for c in range(_C) if c % 2 == 1}
M01_DVE = {c for c in range(_C) if c % 2 == 1}
BUFS_T1 = 3
BUFS_ST = 3
PS_BUFS = (1, 1, 2)               # bufs for (ps1p, psPQ, psRS); sum*2 <= 8

_prog_cache = {}


def _sigmoid32(v):
    v = np.asarray(v, dtype=np.float32)
    return (1.0 / (1.0 + np.exp(-v.astype(np.float64)))).astype(np.float32)


def _gauss1d(k, sigma):
    c = np.arange(k, dtype=np.float64) - k // 2
    g = np.exp(-(c * c) / (2.0 * float(sigma) ** 2))
    return g / g.sum()


def _build_A(k, sigma, H=_H):
    """Combined conv(+resize for even k) operator along one axis (H x H)."""
    pad = k // 2
    Ho = H + 2 * pad - k + 1
    g = _gauss1d(k, sigma)
    S = np.zeros((Ho, H), dtype=np.float64)
    for i in range(Ho):
        lo = max(0, i - pad)
        hi = min(H, i - pad + k)
        for m in range(lo, hi):
            S[i, m] = g[m - i + pad]
    if Ho == H:
        return S.astype(np.float32)
    R = np.zeros((H, Ho), dtype=np.float64)
    scale = Ho / H
    for i in range(H):
        src = (i + 0.5) * scale - 0.5
        i0 = int(np.floor(src))
        t = src - i0
        i0c = min(max(i0, 0), Ho - 1)
        i1c = min(max(i0 + 1, 0), Ho - 1)
        R[i, i0c] += 1.0 - t
        R[i, i1c] += t
    return (R @ S).astype(np.float32)


def _build_program():
    key = ("v3", BATCH_IN, tuple(sorted(PQ_ACT)), tuple(sorted(RS_ACT)),
           tuple(sorted(M01_DVE)), PS_BUFS, BUFS_T1, BUFS_ST)
    if key in _prog_cache:
        return _prog_cache[key]

    from contextlib import ExitStack
    import concourse.bacc as bacc
    import concourse.mybir as mybir
    import concourse.tile as tile

    f32 = mybir.dt.float32
    f16 = mybir.dt.float16
    e3 = mybir.dt.float8e3

    nc = bacc.Bacc(None, target_bir_lowering=False)
    # x prepacked on host: x8[c, p, kc*384 + w] = e3m4(X[c, kc*128 + p, w])
    x_d = nc.declare_dram_parameter("x8", [_C, 128, 3 * _H], e3, isOutput=False)
    at_d = nc.declare_dram_parameter("at", [_H, _H], f16, isOutput=False)
    out_d = nc.declare_dram_parameter("out", [_C, _H, _H], f16, isOutput=True)

    # Banded plan: A^T chunk kc has nonzero cols only in [128*kc-4, 128*kc+132)
    MM_PLAN = [(0, 0, 132, True), (1, 124, 260, False), (2, 252, 384, False)]
    n_mm = len(MM_PLAN)

    def cp(act, dst, src):
        if act:
            nc.scalar.copy(dst, src)
        else:
            nc.vector.tensor_copy(dst, src)

    with tile.TileContext(nc) as tc, ExitStack() as ctx:
        at_pool = ctx.enter_context(tc.tile_pool(name="at", bufs=1))
        x_pool = ctx.enter_context(tc.tile_pool(name="x", bufs=4))
        t1_pool = ctx.enter_context(tc.tile_pool(name="t1", bufs=BUFS_T1))
        st_pool = ctx.enter_context(tc.tile_pool(name="st", bufs=BUFS_ST))
        ps1p = ctx.enter_context(
            tc.tile_pool(name="ps1p", bufs=PS_BUFS[0], space="PSUM"))
        psPQ = ctx.enter_context(
            tc.tile_pool(name="psPQ", bufs=PS_BUFS[1], space="PSUM"))
        psRS = ctx.enter_context(
            tc.tile_pool(name="psRS", bufs=PS_BUFS[2], space="PSUM"))

        at_t = at_pool.tile([128, 3, _H], f16)
        nc.gpsimd.dma_start(
            at_t[:], at_d[:].rearrange("(kc p) i -> p kc i", p=128)
        )

        x_tiles = {}
        pend = {}   # c -> (t1_t, st_t) holding P^T of channel c
        for step in range(_C + 1):
            if step < _C and step % BATCH_IN == 0:
                bi = step // BATCH_IN
                xt = x_pool.tile([128, BATCH_IN, 3, _H], e3, name="xt")
                src = x_d[step : step + BATCH_IN].rearrange(
                    "c p (kc w) -> p c kc w", kc=3
                )
                if bi == 0:
                    for i in range(BATCH_IN):
                        nc.sync.dma_start(xt[:, i], src[:, i])
                else:
                    nc.sync.dma_start(xt[:], src)
                for i in range(BATCH_IN):
                    x_tiles[step + i] = (xt, i)

            c = step            # pass1 channel
            d = step - 1        # pass2 channel
            pq = psPQ.tile([128, 2, 512], f32, name="pq")
            rs = psRS.tile([128, 2, 512], f32, name="rs") if d >= 0 else None
            st_t = st_pool.tile([128, 4, _H], f16, name="stt")

            def mm(dst, lhsT, plan_kc, lo, hi, st_flag, stop_flag):
                nc.tensor.matmul(
                    dst, lhsT, at_t[:, plan_kc, lo:hi],
                    start=st_flag, stop=stop_flag, skip_group_check=True,
                )

            if c < _C:
                xt, xi = x_tiles.pop(c)
                t1_t = t1_pool.tile([128, 2, _H], f16, name="t1t")
                # pass1 m0, m1 -> ps1p pair
                p1p = ps1p.tile([128, 2, 512], f32, name="p1p")
                for m in range(2):
                    for i_mm, (kc, lo, hi, stf) in enumerate(MM_PLAN):
                        mm(p1p[:, m, lo:hi], xt[:, xi, kc, 128 * m : 128 * (m + 1)],
                           kc, lo, hi, stf, i_mm == n_mm - 1)
                # pair-copy m0,m1 — emitted before m2 group
                cp(c not in M01_DVE, t1_t[:], p1p[:, :, 0:_H])
                # pass1 m2 -> pq slot 0
                for i_mm, (kc, lo, hi, stf) in enumerate(MM_PLAN):
                    mm(pq[:, 0, lo:hi], xt[:, xi, kc, 256:384],
                       kc, lo, hi, stf, i_mm == n_mm - 1)
                pend[c] = (t1_t, st_t)

            if d >= 0:
                t1_p, st_p = pend.pop(d)

                def lhs2(kc, it):
                    sl = slice(128 * it, 128 * (it + 1))
                    if kc < 2:
                        return t1_p[:, kc, sl]
                    return st_p[:, 0, sl]

                # it0 -> pq slot 1
                for i_mm, (kc, lo, hi, stf) in enumerate(MM_PLAN):
                    mm(pq[:, 1, lo:hi], lhs2(kc, 0), kc, lo, hi, stf,
                       i_mm == n_mm - 1)
                # pair-copy [pt_m2(c) | O_it0(d)] -> st(c)[:, 0:2]
                if c < _C:
                    cp(c in PQ_ACT, st_t[:, 0:2, :], pq[:, :, 0:_H])
                else:
                    cp(True, st_t[:, 1:2, :], pq[:, 1:2, 0:_H])
                # it1, it2 -> rs
                for it in (1, 2):
                    for i_mm, (kc, lo, hi, stf) in enumerate(MM_PLAN):
                        mm(rs[:, it - 1, lo:hi], lhs2(kc, it), kc, lo, hi, stf,
                           i_mm == n_mm - 1)
                cp(d in RS_ACT, st_t[:, 2:4, :], rs[:, :, 0:_H])
                # output channel d lives in st(c)[:, 1:4]
                nc.gpsimd.dma_start(
                    out_d[d].rearrange("(m p) j -> p m j", p=128),
                    st_t[:, 1:4, :],
                )
            elif c < _C:
                # step 0: no pass2; drain only pt_m2 slot
                cp(True, st_t[:, 0:1, :], pq[:, 0:1, 0:_H])

    nc.finalize()
    _prog_cache[key] = nc
    return nc


def _pack_x(xb, np_e3):
    """xb (64,384,384) f32 -> (64,128,1152) e3m4: [c, p, kc*384+w]."""
    v = xb.reshape(_C, 3, 128, _H).transpose(0, 2, 1, 3).reshape(_C, 128, 3 * _H)
    return np.ascontiguousarray(v.astype(np_e3))


def kernel(x, params, _trace=False):
    from concourse.bass_utils import run_bass_kernel_spmd
    import concourse.mybir as mybir

    x = np.ascontiguousarray(np.asarray(x, dtype=np.float32))
    params = np.asarray(params, dtype=np.float32)
    B = x.shape[0]
    assert x.shape == (_NCORES, _C, _H, _H), x.shape

    k_int = np.trunc(params[:, 0].astype(np.float32))
    k_sel = np.floor(
        np.float32(5.0) + np.float32(5.0) * _sigmoid32(k_int)
    ).astype(np.int32)
    sigma = np.float32(0.5) + np.float32(4.5) * _sigmoid32(params[:, 1])

    np_e3 = mybir.dt.np(mybir.dt.float8e3)

    nc = _build_program()
    in_maps = []
    for b in range(B):
        A = _build_A(int(k_sel[b]), float(sigma[b]))
        at = np.ascontiguousarray(A.T.astype(np.float16))
        in_maps.append({"x8": _pack_x(x[b], np_e3), "at": at})

    res = run_bass_kernel_spmd(nc, in_maps, list(range(_NCORES)), trace=_trace)
    out = np.stack(
        [np.asarray(res.results[b]["out"]).astype(np.float32) for b in range(B)]
    )
    if _trace:
        return out, res
    return out


# revision 4
# speedup vs baseline: 1.0853x; 1.0028x over previous
"""Per-sample Gaussian blur on 8 Trainium2 cores — v3.

Math (as baseline): out_c = A @ X_c @ A^T via two banded tensor-engine
passes; A built on host per sample from (k, sigma).

v3 vs baseline:
- x ships as float8_e3m4 (1 byte), host-prepacked partition-major so DMA
  descriptors are 1152B (full 360GB/s). Mixed-dtype matmul: lhsT=e3
  stationary, rhs=A^T fp16 moving — same 1 cyc/row as fp16. Input DMA
  26us instead of 52us. Measured end-to-end rel_fro error 1.34e-2.
- PSUM repartition: all drains are 2-bank pair copies with >=1-channel
  reuse distance (kills the ps2 single-bank reuse stall of the 2-pool
  layout): ps1p[2]x2 (pass1 m0,m1), psPQ[2]x1 (pass1 m2(c) paired with
  pass2 it0(c-1)), psRS[2]x1 (pass2 it1+it2 of c-1). The PQ pair drains
  into a combined staging tile st(c) = [pt_m2(c) | O_it0..it2(c-1)] so
  one copy serves both passes; output DMA reads st[:, 1:4].
- Copy work balanced across ACT and DVE by assignment tables.
"""

import numpy as np

_H = 384
_C = 64
_NCORES = 8

BATCH_IN = 4         # channels per input DMA instruction
TAIL_ACT = 1         # how many final channels' out-DMAs use the ACT queue

# Engine assignment for the three pair-copies per channel, tuned on the
# cost model: odd channels run m01 on DVE and pq+rs on ACT; even channels
# the mirror image. Alternating the m01 engine decouples the tight
# single-buffered ps1p mm->copy->mm cycle from any one engine's in-order
# queue (96.7us vs 102.6us for fixed assignment).
PQ_ACT = {c for c in range(_C) if c % 2 == 1}
RS_ACT = {c for c in range(_C) if c % 2 == 1}
M01_DVE = {c for c in range(_C) if c % 2 == 1}
BUFS_T1 = 3
BUFS_ST = 3
PS_BUFS = (1, 1, 2)               # bufs for (ps1p, psPQ, psRS); sum*2 <= 8

_prog_cache = {}


def _sigmoid32(v):
    v = np.asarray(v, dtype=np.float32)
    return (1.0 / (1.0 + np.exp(-v.astype(np.float64)))).astype(np.float32)


def _gauss1d(k, sigma):
    c = np.arange(k, dtype=np.float64) - k // 2
    g = np.exp(-(c * c) / (2.0 * float(sigma) ** 2))
    return g / g.sum()


def _build_A(k, sigma, H=_H):
    """Combined conv(+resize for even k) operator along one axis (H x H)."""
    pad = k // 2
    Ho = H + 2 * pad - k + 1
    g = _gauss1d(k, sigma)
    S = np.zeros((Ho, H), dtype=np.float64)
    for i in range(Ho):
        lo = max(0, i - pad)
        hi = min(H, i - pad + k)
        for m in range(lo, hi):
            S[i, m] = g[m - i + pad]
    if Ho == H:
        return S.astype(np.float32)
    R = np.zeros((H, Ho), dtype=np.float64)
    scale = Ho / H
    for i in range(H):
        src = (i + 0.5) * scale - 0.5
        i0 = int(np.floor(src))
        t = src - i0
        i0c = min(max(i0, 0), Ho - 1)
        i1c = min(max(i0 + 1, 0), Ho - 1)
        R[i, i0c] += 1.0 - t
        R[i, i1c] += t
    return (R @ S).astype(np.float32)


def _build_program():
    key = ("v3", TAIL_ACT, BATCH_IN, tuple(sorted(PQ_ACT)), tuple(sorted(RS_ACT)),
           tuple(sorted(M01_DVE)), PS_BUFS, BUFS_T1, BUFS_ST)
    if key in _prog_cache:
        return _prog_cache[key]

    from contextlib import ExitStack
    import concourse.bacc as bacc
    import concourse.mybir as mybir
    import concourse.tile as tile

    f32 = mybir.dt.float32
    f16 = mybir.dt.float16
    e3 = mybir.dt.float8e3

    nc = bacc.Bacc(None, target_bir_lowering=False)
    # x prepacked on host: x8[c, p, kc*384 + w] = e3m4(X[c, kc*128 + p, w])
    x_d = nc.declare_dram_parameter("x8", [_C, 128, 3 * _H], e3, isOutput=False)
    at_d = nc.declare_dram_parameter("at", [_H, _H], f16, isOutput=False)
    out_d = nc.declare_dram_parameter("out", [_C, _H, _H], f16, isOutput=True)

    # Banded plan: A^T chunk kc has nonzero cols only in [128*kc-4, 128*kc+132)
    MM_PLAN = [(0, 0, 132, True), (1, 124, 260, False), (2, 252, 384, False)]
    n_mm = len(MM_PLAN)

    def cp(act, dst, src):
        if act:
            nc.scalar.copy(dst, src)
        else:
            nc.vector.tensor_copy(dst, src)

    with tile.TileContext(nc) as tc, ExitStack() as ctx:
        at_pool = ctx.enter_context(tc.tile_pool(name="at", bufs=1))
        x_pool = ctx.enter_context(tc.tile_pool(name="x", bufs=4))
        t1_pool = ctx.enter_context(tc.tile_pool(name="t1", bufs=BUFS_T1))
        st_pool = ctx.enter_context(tc.tile_pool(name="st", bufs=BUFS_ST))
        ps1p = ctx.enter_context(
            tc.tile_pool(name="ps1p", bufs=PS_BUFS[0], space="PSUM"))
        psPQ = ctx.enter_context(
            tc.tile_pool(name="psPQ", bufs=PS_BUFS[1], space="PSUM"))
        psRS = ctx.enter_context(
            tc.tile_pool(name="psRS", bufs=PS_BUFS[2], space="PSUM"))

        at_t = at_pool.tile([128, 3, _H], f16)
        nc.gpsimd.dma_start(
            at_t[:], at_d[:].rearrange("(kc p) i -> p kc i", p=128)
        )

        x_tiles = {}
        pend = {}   # c -> (t1_t, st_t) holding P^T of channel c
        for step in range(_C + 1):
            if step < _C and step % BATCH_IN == 0:
                bi = step // BATCH_IN
                xt = x_pool.tile([128, BATCH_IN, 3, _H], e3, name="xt")
                src = x_d[step : step + BATCH_IN].rearrange(
                    "c p (kc w) -> p c kc w", kc=3
                )
                if bi == 0:
                    for i in range(BATCH_IN):
                        nc.sync.dma_start(xt[:, i], src[:, i])
                else:
                    nc.sync.dma_start(xt[:], src)
                for i in range(BATCH_IN):
                    x_tiles[step + i] = (xt, i)

            c = step            # pass1 channel
            d = step - 1        # pass2 channel
            pq = psPQ.tile([128, 2, 512], f32, name="pq")
            rs = psRS.tile([128, 2, 512], f32, name="rs") if d >= 0 else None
            st_t = st_pool.tile([128, 4, _H], f16, name="stt")

            def mm(dst, lhsT, plan_kc, lo, hi, st_flag, stop_flag):
                nc.tensor.matmul(
                    dst, lhsT, at_t[:, plan_kc, lo:hi],
                    start=st_flag, stop=stop_flag, skip_group_check=True,
                )

            if c < _C:
                xt, xi = x_tiles.pop(c)
                t1_t = t1_pool.tile([128, 2, _H], f16, name="t1t")
                # pass1 m0, m1 -> ps1p pair
                p1p = ps1p.tile([128, 2, 512], f32, name="p1p")
                for m in range(2):
                    for i_mm, (kc, lo, hi, stf) in enumerate(MM_PLAN):
                        mm(p1p[:, m, lo:hi], xt[:, xi, kc, 128 * m : 128 * (m + 1)],
                           kc, lo, hi, stf, i_mm == n_mm - 1)
                # pair-copy m0,m1 — emitted before m2 group
                cp(c not in M01_DVE, t1_t[:], p1p[:, :, 0:_H])
                # pass1 m2 -> pq slot 0
                for i_mm, (kc, lo, hi, stf) in enumerate(MM_PLAN):
                    mm(pq[:, 0, lo:hi], xt[:, xi, kc, 256:384],
                       kc, lo, hi, stf, i_mm == n_mm - 1)
                pend[c] = (t1_t, st_t)

            if d >= 0:
                t1_p, st_p = pend.pop(d)

                def lhs2(kc, it):
                    sl = slice(128 * it, 128 * (it + 1))
                    if kc < 2:
                        return t1_p[:, kc, sl]
                    return st_p[:, 0, sl]

                # it0 -> pq slot 1
                for i_mm, (kc, lo, hi, stf) in enumerate(MM_PLAN):
                    mm(pq[:, 1, lo:hi], lhs2(kc, 0), kc, lo, hi, stf,
                       i_mm == n_mm - 1)
                # pair-copy [pt_m2(c) | O_it0(d)] -> st(c)[:, 0:2]
                if c < _C:
                    cp(c in PQ_ACT, st_t[:, 0:2, :], pq[:, :, 0:_H])
                else:
                    cp(True, st_t[:, 1:2, :], pq[:, 1:2, 0:_H])
                # it1, it2 -> rs
                for it in (1, 2):
                    for i_mm, (kc, lo, hi, stf) in enumerate(MM_PLAN):
                        mm(rs[:, it - 1, lo:hi], lhs2(kc, it), kc, lo, hi, stf,
                           i_mm == n_mm - 1)
                cp(d in RS_ACT, st_t[:, 2:4, :], rs[:, :, 0:_H])
                # output channel d lives in st(c)[:, 1:4]
                odst = out_d[d].rearrange("(m p) j -> p m j", p=128)
                if d >= _C - TAIL_ACT:
                    # tail: ACT's HWDGE queue is idle after its last copy and
                    # generates descriptors ~0.6us faster than Pool SWDGE
                    nc.scalar.dma_start(odst, st_t[:, 1:4, :])
                else:
                    nc.gpsimd.dma_start(odst, st_t[:, 1:4, :])
            elif c < _C:
                # step 0: no pass2; drain only pt_m2 slot
                cp(True, st_t[:, 0:1, :], pq[:, 0:1, 0:_H])

    nc.finalize()
    _prog_cache[key] = nc
    return nc


def _pack_x(xb, np_e3):
    """xb (64,384,384) f32 -> (64,128,1152) e3m4: [c, p, kc*384+w]."""
    v = xb.reshape(_C, 3, 128, _H).transpose(0, 2, 1, 3).reshape(_C, 128, 3 * _H)
    return np.ascontiguousarray(v.astype(np_e3))


def kernel(x, params, _trace=False):
    from concourse.bass_utils import run_bass_kernel_spmd
    import concourse.mybir as mybir

    x = np.ascontiguousarray(np.asarray(x, dtype=np.float32))
    params = np.asarray(params, dtype=np.float32)
    B = x.shape[0]
    assert x.shape == (_NCORES, _C, _H, _H), x.shape

    k_int = np.trunc(params[:, 0].astype(np.float32))
    k_sel = np.floor(
        np.float32(5.0) + np.float32(5.0) * _sigmoid32(k_int)
    ).astype(np.int32)
    sigma = np.float32(0.5) + np.float32(4.5) * _sigmoid32(params[:, 1])

    np_e3 = mybir.dt.np(mybir.dt.float8e3)

    nc = _build_program()
    in_maps = []
    for b in range(B):
        A = _build_A(int(k_sel[b]), float(sigma[b]))
        at = np.ascontiguousarray(A.T.astype(np.float16))
        in_maps.append({"x8": _pack_x(x[b], np_e3), "at": at})

    res = run_bass_kernel_spmd(nc, in_maps, list(range(_NCORES)), trace=_trace)
    out = np.stack(
        [np.asarray(res.results[b]["out"]).astype(np.float32) for b in range(B)]
    )
    if _trace:
        return out, res
    return out


# revision 5
# speedup vs baseline: 1.0885x; 1.0029x over previous
"""Per-sample Gaussian blur on 8 Trainium2 cores — v3.

Math (as baseline): out_c = A @ X_c @ A^T via two banded tensor-engine
passes; A built on host per sample from (k, sigma).

v3 vs baseline:
- x ships as float8_e3m4 (1 byte), host-prepacked partition-major so DMA
  descriptors are 1152B (full 360GB/s). Mixed-dtype matmul: lhsT=e3
  stationary, rhs=A^T fp16 moving — same 1 cyc/row as fp16. Input DMA
  26us instead of 52us. Measured end-to-end rel_fro error 1.34e-2.
- PSUM repartition: all drains are 2-bank pair copies with >=1-channel
  reuse distance (kills the ps2 single-bank reuse stall of the 2-pool
  layout): ps1p[2]x2 (pass1 m0,m1), psPQ[2]x1 (pass1 m2(c) paired with
  pass2 it0(c-1)), psRS[2]x1 (pass2 it1+it2 of c-1). The PQ pair drains
  into a combined staging tile st(c) = [pt_m2(c) | O_it0..it2(c-1)] so
  one copy serves both passes; output DMA reads st[:, 1:4].
- Copy work balanced across ACT and DVE by assignment tables.
"""

import numpy as np

_H = 384
_C = 64
_NCORES = 8

BATCH_IN = 4         # channels per input DMA instruction
TAIL_ACT = 1         # how many final channels' out-DMAs use the ACT queue

# Engine assignment for the three pair-copies per channel, tuned on the
# cost model: odd channels run m01 on DVE and pq+rs on ACT; even channels
# the mirror image. Alternating the m01 engine decouples the tight
# single-buffered ps1p mm->copy->mm cycle from any one engine's in-order
# queue (96.7us vs 102.6us for fixed assignment).
PQ_ACT = {c for c in range(_C) if c % 2 == 1}
RS_ACT = {c for c in range(_C) if c % 2 == 1}
M01_DVE = {c for c in range(_C) if c % 2 == 1}
BUFS_T1 = 3
BUFS_ST = 3
PS_BUFS = (1, 1, 2)               # bufs for (ps1p, psPQ, psRS); sum*2 <= 8

_prog_cache = {}


def _sigmoid32(v):
    v = np.asarray(v, dtype=np.float32)
    return (1.0 / (1.0 + np.exp(-v.astype(np.float64)))).astype(np.float32)


def _gauss1d(k, sigma):
    c = np.arange(k, dtype=np.float64) - k // 2
    g = np.exp(-(c * c) / (2.0 * float(sigma) ** 2))
    return g / g.sum()


def _build_A(k, sigma, H=_H):
    """Combined conv(+resize for even k) operator along one axis (H x H)."""
    pad = k // 2
    Ho = H + 2 * pad - k + 1
    g = _gauss1d(k, sigma)
    S = np.zeros((Ho, H), dtype=np.float64)
    for i in range(Ho):
        lo = max(0, i - pad)
        hi = min(H, i - pad + k)
        for m in range(lo, hi):
            S[i, m] = g[m - i + pad]
    if Ho == H:
        return S.astype(np.float32)
    R = np.zeros((H, Ho), dtype=np.float64)
    scale = Ho / H
    for i in range(H):
        src = (i + 0.5) * scale - 0.5
        i0 = int(np.floor(src))
        t = src - i0
        i0c = min(max(i0, 0), Ho - 1)
        i1c = min(max(i0 + 1, 0), Ho - 1)
        R[i, i0c] += 1.0 - t
        R[i, i1c] += t
    return (R @ S).astype(np.float32)


def _build_program():
    key = ("v3", TAIL_ACT, BATCH_IN, tuple(sorted(PQ_ACT)), tuple(sorted(RS_ACT)),
           tuple(sorted(M01_DVE)), PS_BUFS, BUFS_T1, BUFS_ST)
    if key in _prog_cache:
        return _prog_cache[key]

    from contextlib import ExitStack
    import concourse.bacc as bacc
    import concourse.mybir as mybir
    import concourse.tile as tile

    f32 = mybir.dt.float32
    f16 = mybir.dt.float16
    e3 = mybir.dt.float8e3

    nc = bacc.Bacc(None, target_bir_lowering=False)
    # x prepacked on host: x8[c, p, kc*384 + w] = e3m4(X[c, kc*128 + p, w])
    x_d = nc.declare_dram_parameter("x8", [_C, 128, 3 * _H], e3, isOutput=False)
    at_d = nc.declare_dram_parameter("at", [_H, _H], f16, isOutput=False)
    out_d = nc.declare_dram_parameter("out", [_C, _H, _H], f16, isOutput=True)

    # Banded plan: A^T chunk kc has nonzero cols only in [128*kc-4, 128*kc+132)
    MM_PLAN = [(0, 0, 132, True), (1, 124, 260, False), (2, 252, 384, False)]
    n_mm = len(MM_PLAN)

    def cp(act, dst, src):
        if act:
            nc.scalar.copy(dst, src)
        else:
            nc.vector.tensor_copy(dst, src)

    with tile.TileContext(nc) as tc, ExitStack() as ctx:
        at_pool = ctx.enter_context(tc.tile_pool(name="at", bufs=1))
        x_pool = ctx.enter_context(tc.tile_pool(name="x", bufs=4))
        t1_pool = ctx.enter_context(tc.tile_pool(name="t1", bufs=BUFS_T1))
        st_pool = ctx.enter_context(tc.tile_pool(name="st", bufs=BUFS_ST))
        ps1p = ctx.enter_context(
            tc.tile_pool(name="ps1p", bufs=PS_BUFS[0], space="PSUM"))
        psPQ = ctx.enter_context(
            tc.tile_pool(name="psPQ", bufs=PS_BUFS[1], space="PSUM"))
        psRS = ctx.enter_context(
            tc.tile_pool(name="psRS", bufs=PS_BUFS[2], space="PSUM"))

        at_t = at_pool.tile([128, 3, _H], f16)
        nc.gpsimd.dma_start(
            at_t[:], at_d[:].rearrange("(kc p) i -> p kc i", p=128)
        )

        x_tiles = {}
        pend = {}   # c -> (t1_t, st_t) holding P^T of channel c
        for step in range(_C + 1):
            if step < _C and step % BATCH_IN == 0:
                bi = step // BATCH_IN
                xt = x_pool.tile([128, BATCH_IN, 3, _H], e3, name="xt")
                src = x_d[step : step + BATCH_IN].rearrange(
                    "c p (kc w) -> p c kc w", kc=3
                )
                if bi == 0:
                    for i in range(BATCH_IN):
                        nc.sync.dma_start(xt[:, i], src[:, i])
                else:
                    nc.sync.dma_start(xt[:], src)
                for i in range(BATCH_IN):
                    x_tiles[step + i] = (xt, i)

            c = step            # pass1 channel
            d = step - 1        # pass2 channel
            pq = psPQ.tile([128, 2, 512], f32, name="pq")
            rs = psRS.tile([128, 2, 512], f32, name="rs") if d >= 0 else None
            st_t = st_pool.tile([128, 4, _H], f16, name="stt")

            def mm(dst, lhsT, plan_kc, lo, hi, st_flag, stop_flag):
                nc.tensor.matmul(
                    dst, lhsT, at_t[:, plan_kc, lo:hi],
                    start=st_flag, stop=stop_flag, skip_group_check=True,
                )

            if c < _C:
                xt, xi = x_tiles.pop(c)
                t1_t = t1_pool.tile([128, 2, _H], f16, name="t1t")
                # pass1 m0, m1 -> ps1p pair
                p1p = ps1p.tile([128, 2, 512], f32, name="p1p")
                for m in range(2):
                    for i_mm, (kc, lo, hi, stf) in enumerate(MM_PLAN):
                        mm(p1p[:, m, lo:hi], xt[:, xi, kc, 128 * m : 128 * (m + 1)],
                           kc, lo, hi, stf, i_mm == n_mm - 1)
                # pair-copy m0,m1 — emitted before m2 group
                cp(c not in M01_DVE, t1_t[:], p1p[:, :, 0:_H])
                # pass1 m2 -> pq slot 0
                for i_mm, (kc, lo, hi, stf) in enumerate(MM_PLAN):
                    mm(pq[:, 0, lo:hi], xt[:, xi, kc, 256:384],
                       kc, lo, hi, stf, i_mm == n_mm - 1)
                pend[c] = (t1_t, st_t)

            if d >= 0:
                t1_p, st_p = pend.pop(d)

                def lhs2(kc, it):
                    sl = slice(128 * it, 128 * (it + 1))
                    if kc < 2:
                        return t1_p[:, kc, sl]
                    return st_p[:, 0, sl]

                # it0 -> pq slot 1
                for i_mm, (kc, lo, hi, stf) in enumerate(MM_PLAN):
                    mm(pq[:, 1, lo:hi], lhs2(kc, 0), kc, lo, hi, stf,
                       i_mm == n_mm - 1)
                # pair-copy [pt_m2(c) | O_it0(d)] -> st(c)[:, 0:2]
                if c < _C:
                    cp(c in PQ_ACT, st_t[:, 0:2, :], pq[:, :, 0:_H])
                else:
                    # final step: DVE drains it0(63) while ACT runs rs(63),
                    # so the tail DMA's deps complete in parallel
                    cp(False, st_t[:, 1:2, :], pq[:, 1:2, 0:_H])
                # it1, it2 -> rs
                for it in (1, 2):
                    for i_mm, (kc, lo, hi, stf) in enumerate(MM_PLAN):
                        mm(rs[:, it - 1, lo:hi], lhs2(kc, it), kc, lo, hi, stf,
                           i_mm == n_mm - 1)
                cp(d in RS_ACT, st_t[:, 2:4, :], rs[:, :, 0:_H])
                # output channel d lives in st(c)[:, 1:4]
                odst = out_d[d].rearrange("(m p) j -> p m j", p=128)
                if d >= _C - TAIL_ACT:
                    # tail: ACT's HWDGE queue is idle after its last copy and
                    # generates descriptors ~0.6us faster than Pool SWDGE
                    nc.scalar.dma_start(odst, st_t[:, 1:4, :])
                else:
                    nc.gpsimd.dma_start(odst, st_t[:, 1:4, :])
            elif c < _C:
                # step 0: no pass2; drain only pt_m2 slot
                cp(True, st_t[:, 0:1, :], pq[:, 0:1, 0:_H])

    nc.finalize()
    _prog_cache[key] = nc
    return nc


def _pack_x(xb, np_e3):
    """xb (64,384,384) f32 -> (64,128,1152) e3m4: [c, p, kc*384+w]."""
    v = xb.reshape(_C, 3, 128, _H).transpose(0, 2, 1, 3).reshape(_C, 128, 3 * _H)
    return np.ascontiguousarray(v.astype(np_e3))


def kernel(x, params, _trace=False):
    from concourse.bass_utils import run_bass_kernel_spmd
    import concourse.mybir as mybir

    x = np.ascontiguousarray(np.asarray(x, dtype=np.float32))
    params = np.asarray(params, dtype=np.float32)
    B = x.shape[0]
    assert x.shape == (_NCORES, _C, _H, _H), x.shape

    k_int = np.trunc(params[:, 0].astype(np.float32))
    k_sel = np.floor(
        np.float32(5.0) + np.float32(5.0) * _sigmoid32(k_int)
    ).astype(np.int32)
    sigma = np.float32(0.5) + np.float32(4.5) * _sigmoid32(params[:, 1])

    np_e3 = mybir.dt.np(mybir.dt.float8e3)

    nc = _build_program()
    in_maps = []
    for b in range(B):
        A = _build_A(int(k_sel[b]), float(sigma[b]))
        at = np.ascontiguousarray(A.T.astype(np.float16))
        in_maps.append({"x8": _pack_x(x[b], np_e3), "at": at})

    res = run_bass_kernel_spmd(nc, in_maps, list(range(_NCORES)), trace=_trace)
    out = np.stack(
        [np.asarray(res.results[b]["out"]).astype(np.float32) for b in range(B)]
    )
    if _trace:
        return out, res
    return out


# revision 6
# speedup vs baseline: 1.0888x; 1.0002x over previous
"""Per-sample Gaussian blur on 8 Trainium2 cores — v3.

Math (as baseline): out_c = A @ X_c @ A^T via two banded tensor-engine
passes; A built on host per sample from (k, sigma).

v3 vs baseline:
- x ships as float8_e3m4 (1 byte), host-prepacked partition-major so DMA
  descriptors are 1152B (full 360GB/s). Mixed-dtype matmul: lhsT=e3
  stationary, rhs=A^T fp16 moving — same 1 cyc/row as fp16. Input DMA
  26us instead of 52us. Measured end-to-end rel_fro error 1.34e-2.
- PSUM repartition: all drains are 2-bank pair copies with >=1-channel
  reuse distance (kills the ps2 single-bank reuse stall of the 2-pool
  layout): ps1p[2]x2 (pass1 m0,m1), psPQ[2]x1 (pass1 m2(c) paired with
  pass2 it0(c-1)), psRS[2]x1 (pass2 it1+it2 of c-1). The PQ pair drains
  into a combined staging tile st(c) = [pt_m2(c) | O_it0..it2(c-1)] so
  one copy serves both passes; output DMA reads st[:, 1:4].
- Copy work balanced across ACT and DVE by assignment tables.
"""

import numpy as np

_H = 384
_C = 64
_NCORES = 8

BATCH_IN = 4         # channels per input DMA instruction
TAIL_ACT = 1         # how many final channels' out-DMAs use the ACT queue

# Engine assignment for the three pair-copies per channel, tuned on the
# cost model: odd channels run m01 on DVE and pq+rs on ACT; even channels
# the mirror image. Alternating the m01 engine decouples the tight
# single-buffered ps1p mm->copy->mm cycle from any one engine's in-order
# queue (96.7us vs 102.6us for fixed assignment).
PQ_ACT = {c for c in range(_C) if c % 2 == 1}
RS_ACT = {c for c in range(_C) if c % 2 == 1}
M01_DVE = {c for c in range(_C) if c % 2 == 1}
BUFS_T1 = 3
BUFS_ST = 3
PS_BUFS = (1, 1, 2)               # bufs for (ps1p, psPQ, psRS); sum*2 <= 8

_prog_cache = {}


def _sigmoid32(v):
    v = np.asarray(v, dtype=np.float32)
    return (1.0 / (1.0 + np.exp(-v.astype(np.float64)))).astype(np.float32)


def _gauss1d(k, sigma):
    c = np.arange(k, dtype=np.float64) - k // 2
    g = np.exp(-(c * c) / (2.0 * float(sigma) ** 2))
    return g / g.sum()


def _build_A(k, sigma, H=_H):
    """Combined conv(+resize for even k) operator along one axis (H x H)."""
    pad = k // 2
    Ho = H + 2 * pad - k + 1
    g = _gauss1d(k, sigma)
    S = np.zeros((Ho, H), dtype=np.float64)
    for i in range(Ho):
        lo = max(0, i - pad)
        hi = min(H, i - pad + k)
        for m in range(lo, hi):
            S[i, m] = g[m - i + pad]
    if Ho == H:
        return S.astype(np.float32)
    R = np.zeros((H, Ho), dtype=np.float64)
    scale = Ho / H
    for i in range(H):
        src = (i + 0.5) * scale - 0.5
        i0 = int(np.floor(src))
        t = src - i0
        i0c = min(max(i0, 0), Ho - 1)
        i1c = min(max(i0 + 1, 0), Ho - 1)
        R[i, i0c] += 1.0 - t
        R[i, i1c] += t
    return (R @ S).astype(np.float32)


def _build_program():
    key = ("v3", TAIL_ACT, BATCH_IN, tuple(sorted(PQ_ACT)), tuple(sorted(RS_ACT)),
           tuple(sorted(M01_DVE)), PS_BUFS, BUFS_T1, BUFS_ST)
    if key in _prog_cache:
        return _prog_cache[key]

    from contextlib import ExitStack
    import concourse.bacc as bacc
    import concourse.mybir as mybir
    import concourse.tile as tile

    f32 = mybir.dt.float32
    f16 = mybir.dt.float16
    e3 = mybir.dt.float8e3

    nc = bacc.Bacc(None, target_bir_lowering=False)
    # x prepacked on host: x8[c, p, kc*384 + w] = e3m4(X[c, kc*128 + p, w])
    x_d = nc.declare_dram_parameter("x8", [_C, 128, 3 * _H], e3, isOutput=False)
    at_d = nc.declare_dram_parameter("at", [_H, _H], f16, isOutput=False)
    out_d = nc.declare_dram_parameter("out", [_C, _H, _H], f16, isOutput=True)

    # Banded plan: A^T chunk kc has nonzero cols only in [128*kc-4, 128*kc+132)
    MM_PLAN = [(0, 0, 132, True), (1, 124, 260, False), (2, 252, 384, False)]
    n_mm = len(MM_PLAN)

    def cp(act, dst, src):
        if act:
            nc.scalar.copy(dst, src)
        else:
            nc.vector.tensor_copy(dst, src)

    with tile.TileContext(nc) as tc, ExitStack() as ctx:
        at_pool = ctx.enter_context(tc.tile_pool(name="at", bufs=1))
        x_pool = ctx.enter_context(tc.tile_pool(name="x", bufs=4))
        t1_pool = ctx.enter_context(tc.tile_pool(name="t1", bufs=BUFS_T1))
        st_pool = ctx.enter_context(tc.tile_pool(name="st", bufs=BUFS_ST))
        ps1p = ctx.enter_context(
            tc.tile_pool(name="ps1p", bufs=PS_BUFS[0], space="PSUM"))
        psPQ = ctx.enter_context(
            tc.tile_pool(name="psPQ", bufs=PS_BUFS[1], space="PSUM"))
        psRS = ctx.enter_context(
            tc.tile_pool(name="psRS", bufs=PS_BUFS[2], space="PSUM"))

        at_t = at_pool.tile([128, 3, _H], f16)
        nc.gpsimd.dma_start(
            at_t[:], at_d[:].rearrange("(kc p) i -> p kc i", p=128)
        )

        x_tiles = {}
        pend = {}   # c -> (t1_t, st_t) holding P^T of channel c
        for step in range(_C + 1):
            if step < _C and step % BATCH_IN == 0:
                bi = step // BATCH_IN
                xt = x_pool.tile([128, BATCH_IN, 3, _H], e3, name="xt")
                src = x_d[step : step + BATCH_IN].rearrange(
                    "c p (kc w) -> p c kc w", kc=3
                )
                if bi == 0:
                    for i in range(BATCH_IN):
                        nc.sync.dma_start(xt[:, i], src[:, i])
                else:
                    nc.sync.dma_start(xt[:], src)
                for i in range(BATCH_IN):
                    x_tiles[step + i] = (xt, i)

            c = step            # pass1 channel
            d = step - 1        # pass2 channel
            pq = psPQ.tile([128, 2, 512], f32, name="pq")
            rs = psRS.tile([128, 2, 512], f32, name="rs") if d >= 0 else None
            st_t = st_pool.tile([128, 4, _H], f16, name="stt")

            def mm(dst, lhsT, plan_kc, lo, hi, st_flag, stop_flag):
                nc.tensor.matmul(
                    dst, lhsT, at_t[:, plan_kc, lo:hi],
                    start=st_flag, stop=stop_flag, skip_group_check=True,
                )

            if c < _C:
                xt, xi = x_tiles.pop(c)
                t1_t = t1_pool.tile([128, 2, _H], f16, name="t1t")
                # pass1 m0, m1 -> ps1p pair
                p1p = ps1p.tile([128, 2, 512], f32, name="p1p")
                for m in range(2):
                    for i_mm, (kc, lo, hi, stf) in enumerate(MM_PLAN):
                        mm(p1p[:, m, lo:hi], xt[:, xi, kc, 128 * m : 128 * (m + 1)],
                           kc, lo, hi, stf, i_mm == n_mm - 1)
                # pair-copy m0,m1 — emitted before m2 group
                cp(c not in M01_DVE, t1_t[:], p1p[:, :, 0:_H])
                # pass1 m2 -> pq slot 0
                for i_mm, (kc, lo, hi, stf) in enumerate(MM_PLAN):
                    mm(pq[:, 0, lo:hi], xt[:, xi, kc, 256:384],
                       kc, lo, hi, stf, i_mm == n_mm - 1)
                pend[c] = (t1_t, st_t)

            if d >= 0:
                t1_p, st_p = pend.pop(d)

                def lhs2(kc, it):
                    sl = slice(128 * it, 128 * (it + 1))
                    if kc < 2:
                        return t1_p[:, kc, sl]
                    return st_p[:, 0, sl]

                # it0 -> pq slot 1
                for i_mm, (kc, lo, hi, stf) in enumerate(MM_PLAN):
                    mm(pq[:, 1, lo:hi], lhs2(kc, 0), kc, lo, hi, stf,
                       i_mm == n_mm - 1)
                # pair-copy [pt_m2(c) | O_it0(d)] -> st(c)[:, 0:2]
                if c < _C:
                    cp(c in PQ_ACT, st_t[:, 0:2, :], pq[:, :, 0:_H])
                else:
                    # final step: DVE drains it0(63) while ACT runs rs(63),
                    # so the tail DMA's deps complete in parallel
                    cp(False, st_t[:, 1:2, :], pq[:, 1:2, 0:_H])
                # it1, it2 -> rs
                for it in (1, 2):
                    for i_mm, (kc, lo, hi, stf) in enumerate(MM_PLAN):
                        mm(rs[:, it - 1, lo:hi], lhs2(kc, it), kc, lo, hi, stf,
                           i_mm == n_mm - 1)
                cp(d in RS_ACT, st_t[:, 2:4, :], rs[:, :, 0:_H])
                # output channel d lives in st(c)[:, 1:4]
                odst = out_d[d].rearrange("(m p) j -> p m j", p=128)
                if d >= _C - TAIL_ACT:
                    # tail: ACT's HWDGE queue is idle after its last copy and
                    # generates descriptors ~0.6us faster than Pool SWDGE
                    nc.scalar.dma_start(odst, st_t[:, 1:4, :])
                else:
                    nc.gpsimd.dma_start(odst, st_t[:, 1:4, :])
            elif c < _C:
                # step 0: no pass2; drain pt_m2 on idle DVE so it runs in
                # parallel with ACT's m01 copy (both gate pass2 of channel 0)
                cp(False, st_t[:, 0:1, :], pq[:, 0:1, 0:_H])

    nc.finalize()
    _prog_cache[key] = nc
    return nc


def _pack_x(xb, np_e3):
    """xb (64,384,384) f32 -> (64,128,1152) e3m4: [c, p, kc*384+w]."""
    v = xb.reshape(_C, 3, 128, _H).transpose(0, 2, 1, 3).reshape(_C, 128, 3 * _H)
    return np.ascontiguousarray(v.astype(np_e3))


def kernel(x, params, _trace=False):
    from concourse.bass_utils import run_bass_kernel_spmd
    import concourse.mybir as mybir

    x = np.ascontiguousarray(np.asarray(x, dtype=np.float32))
    params = np.asarray(params, dtype=np.float32)
    B = x.shape[0]
    assert x.shape == (_NCORES, _C, _H, _H), x.shape

    k_int = np.trunc(params[:, 0].astype(np.float32))
    k_sel = np.floor(
        np.float32(5.0) + np.float32(5.0) * _sigmoid32(k_int)
    ).astype(np.int32)
    sigma = np.float32(0.5) + np.float32(4.5) * _sigmoid32(params[:, 1])

    np_e3 = mybir.dt.np(mybir.dt.float8e3)

    nc = _build_program()
    in_maps = []
    for b in range(B):
        A = _build_A(int(k_sel[b]), float(sigma[b]))
        at = np.ascontiguousarray(A.T.astype(np.float16))
        in_maps.append({"x8": _pack_x(x[b], np_e3), "at": at})

    res = run_bass_kernel_spmd(nc, in_maps, list(range(_NCORES)), trace=_trace)
    out = np.stack(
        [np.asarray(res.results[b]["out"]).astype(np.float32) for b in range(B)]
    )
    if _trace:
        return out, res
    return out


# revision 8
# speedup vs baseline: 1.0987x; 1.0091x over previous
"""Per-sample Gaussian blur on 8 Trainium2 cores — v3.

Math (as baseline): out_c = A @ X_c @ A^T via two banded tensor-engine
passes; A built on host per sample from (k, sigma).

v3 vs baseline:
- x ships as float8_e3m4 (1 byte), host-prepacked partition-major so DMA
  descriptors are 1152B (full 360GB/s). Mixed-dtype matmul: lhsT=e3
  stationary, rhs=A^T fp16 moving — same 1 cyc/row as fp16. Input DMA
  26us instead of 52us. Measured end-to-end rel_fro error 1.34e-2.
- PSUM repartition: all drains are 2-bank pair copies with >=1-channel
  reuse distance (kills the ps2 single-bank reuse stall of the 2-pool
  layout): ps1p[2]x2 (pass1 m0,m1), psPQ[2]x1 (pass1 m2(c) paired with
  pass2 it0(c-1)), psRS[2]x1 (pass2 it1+it2 of c-1). The PQ pair drains
  into a combined staging tile st(c) = [pt_m2(c) | O_it0..it2(c-1)] so
  one copy serves both passes; output DMA reads st[:, 1:4].
- Copy work balanced across ACT and DVE by assignment tables.
"""

import numpy as np

_H = 384
_C = 64
_NCORES = 8

BATCH_IN = 4         # channels per input DMA instruction
TAIL_ACT = 1         # how many final channels' out-DMAs use the ACT queue

# Engine assignment for the three pair-copies per channel, tuned on the
# cost model: odd channels run m01 on DVE and pq+rs on ACT; even channels
# the mirror image. Alternating the m01 engine decouples the tight
# single-buffered ps1p mm->copy->mm cycle from any one engine's in-order
# queue (96.7us vs 102.6us for fixed assignment).
PQ_ACT = {c for c in range(_C) if c % 2 == 1} | {10, 20, 30, 42, 52, 62}
RS_ACT = {c for c in range(_C) if c % 2 == 1}
M01_DVE = {c for c in range(_C) if c % 2 == 1}
BUFS_T1 = 3
BUFS_ST = 3
PS_BUFS = (1, 1, 2)               # bufs for (ps1p, psPQ, psRS); sum*2 <= 8

_prog_cache = {}


def _sigmoid32(v):
    v = np.asarray(v, dtype=np.float32)
    return (1.0 / (1.0 + np.exp(-v.astype(np.float64)))).astype(np.float32)


def _gauss1d(k, sigma):
    c = np.arange(k, dtype=np.float64) - k // 2
    g = np.exp(-(c * c) / (2.0 * float(sigma) ** 2))
    return g / g.sum()


def _build_A(k, sigma, H=_H):
    """Combined conv(+resize for even k) operator along one axis (H x H)."""
    pad = k // 2
    Ho = H + 2 * pad - k + 1
    g = _gauss1d(k, sigma)
    S = np.zeros((Ho, H), dtype=np.float64)
    for i in range(Ho):
        lo = max(0, i - pad)
        hi = min(H, i - pad + k)
        for m in range(lo, hi):
            S[i, m] = g[m - i + pad]
    if Ho == H:
        return S.astype(np.float32)
    R = np.zeros((H, Ho), dtype=np.float64)
    scale = Ho / H
    for i in range(H):
        src = (i + 0.5) * scale - 0.5
        i0 = int(np.floor(src))
        t = src - i0
        i0c = min(max(i0, 0), Ho - 1)
        i1c = min(max(i0 + 1, 0), Ho - 1)
        R[i, i0c] += 1.0 - t
        R[i, i1c] += t
    return (R @ S).astype(np.float32)


def _build_program():
    key = ("v3", TAIL_ACT, BATCH_IN, tuple(sorted(PQ_ACT)), tuple(sorted(RS_ACT)),
           tuple(sorted(M01_DVE)), PS_BUFS, BUFS_T1, BUFS_ST)
    if key in _prog_cache:
        return _prog_cache[key]

    from contextlib import ExitStack
    import concourse.bacc as bacc
    import concourse.mybir as mybir
    import concourse.tile as tile

    f32 = mybir.dt.float32
    f16 = mybir.dt.float16
    e3 = mybir.dt.float8e3

    nc = bacc.Bacc(None, target_bir_lowering=False)
    # x prepacked on host: x8[c, p, kc*384 + w] = e3m4(X[c, kc*128 + p, w])
    x_d = nc.declare_dram_parameter("x8", [_C, 128, 3 * _H], e3, isOutput=False)
    at_d = nc.declare_dram_parameter("at", [_H, _H], f16, isOutput=False)
    out_d = nc.declare_dram_parameter("out", [_C, _H, _H], f16, isOutput=True)

    # Banded plan: A^T chunk kc has nonzero cols only in [128*kc-4, 128*kc+132)
    MM_PLAN = [(0, 0, 132, True), (1, 124, 260, False), (2, 252, 384, False)]
    n_mm = len(MM_PLAN)

    def cp(act, dst, src):
        if act:
            nc.scalar.copy(dst, src)
        else:
            nc.vector.tensor_copy(dst, src)

    with tile.TileContext(nc) as tc, ExitStack() as ctx:
        at_pool = ctx.enter_context(tc.tile_pool(name="at", bufs=1))
        x_pool = ctx.enter_context(tc.tile_pool(name="x", bufs=4))
        t1_pool = ctx.enter_context(tc.tile_pool(name="t1", bufs=BUFS_T1))
        st_pool = ctx.enter_context(tc.tile_pool(name="st", bufs=BUFS_ST))
        psALL = ctx.enter_context(
            tc.tile_pool(name="psALL", bufs=4, space="PSUM"))
        ps1p = psPQ = psRS = psALL

        at_t = at_pool.tile([128, 3, _H], f16)
        nc.gpsimd.dma_start(
            at_t[:], at_d[:].rearrange("(kc p) i -> p kc i", p=128)
        )

        x_tiles = {}
        pend = {}   # c -> (t1_t, st_t) holding P^T of channel c
        for step in range(_C + 1):
            if step < _C and step % BATCH_IN == 0:
                bi = step // BATCH_IN
                xt = x_pool.tile([128, BATCH_IN, 3, _H], e3, name="xt")
                src = x_d[step : step + BATCH_IN].rearrange(
                    "c p (kc w) -> p c kc w", kc=3
                )
                if bi == 0:
                    for i in range(BATCH_IN):
                        nc.sync.dma_start(xt[:, i], src[:, i])
                else:
                    nc.sync.dma_start(xt[:], src)
                for i in range(BATCH_IN):
                    x_tiles[step + i] = (xt, i)

            c = step            # pass1 channel
            d = step - 1        # pass2 channel
            st_t = st_pool.tile([128, 4, _H], f16, name="stt")
            pq = None
            rs = None

            def mm(dst, lhsT, plan_kc, lo, hi, st_flag, stop_flag):
                nc.tensor.matmul(
                    dst, lhsT, at_t[:, plan_kc, lo:hi],
                    start=st_flag, stop=stop_flag, skip_group_check=True,
                )

            if c < _C:
                xt, xi = x_tiles.pop(c)
                t1_t = t1_pool.tile([128, 2, _H], f16, name="t1t")
                # pass1 m0, m1 -> ps1p pair
                p1p = ps1p.tile([128, 2, 512], f32, name="pspair")
                for m in range(2):
                    for i_mm, (kc, lo, hi, stf) in enumerate(MM_PLAN):
                        mm(p1p[:, m, lo:hi], xt[:, xi, kc, 128 * m : 128 * (m + 1)],
                           kc, lo, hi, stf, i_mm == n_mm - 1)
                # pair-copy m0,m1 — emitted before m2 group
                cp(c not in M01_DVE, t1_t[:], p1p[:, :, 0:_H])
                pq = psALL.tile([128, 2, 512], f32, name="pspair")
                # pass1 m2 -> pq slot 0
                for i_mm, (kc, lo, hi, stf) in enumerate(MM_PLAN):
                    mm(pq[:, 0, lo:hi], xt[:, xi, kc, 256:384],
                       kc, lo, hi, stf, i_mm == n_mm - 1)
                pend[c] = (t1_t, st_t)

            if d >= 0:
                if pq is None:
                    pq = psALL.tile([128, 2, 512], f32, name="pspair")
                rs = psALL.tile([128, 2, 512], f32, name="pspair")
                t1_p, st_p = pend.pop(d)

                def lhs2(kc, it):
                    sl = slice(128 * it, 128 * (it + 1))
                    if kc < 2:
                        return t1_p[:, kc, sl]
                    return st_p[:, 0, sl]

                # it0 -> pq slot 1
                for i_mm, (kc, lo, hi, stf) in enumerate(MM_PLAN):
                    mm(pq[:, 1, lo:hi], lhs2(kc, 0), kc, lo, hi, stf,
                       i_mm == n_mm - 1)
                # pair-copy [pt_m2(c) | O_it0(d)] -> st(c)[:, 0:2]
                if c < _C:
                    cp(c in PQ_ACT, st_t[:, 0:2, :], pq[:, :, 0:_H])
                else:
                    # final step: DVE drains it0(63) while ACT runs rs(63),
                    # so the tail DMA's deps complete in parallel
                    cp(False, st_t[:, 1:2, :], pq[:, 1:2, 0:_H])
                # it1, it2 -> rs
                for it in (1, 2):
                    for i_mm, (kc, lo, hi, stf) in enumerate(MM_PLAN):
                        mm(rs[:, it - 1, lo:hi], lhs2(kc, it), kc, lo, hi, stf,
                           i_mm == n_mm - 1)
                cp(d in RS_ACT, st_t[:, 2:4, :], rs[:, :, 0:_H])
                # output channel d lives in st(c)[:, 1:4]
                odst = out_d[d].rearrange("(m p) j -> p m j", p=128)
                if d >= _C - TAIL_ACT:
                    # tail: ACT's HWDGE queue is idle after its last copy and
                    # generates descriptors ~0.6us faster than Pool SWDGE
                    nc.scalar.dma_start(odst, st_t[:, 1:4, :])
                else:
                    nc.gpsimd.dma_start(odst, st_t[:, 1:4, :])
            elif c < _C:
                # step 0: no pass2; drain pt_m2 on idle DVE so it runs in
                # parallel with ACT's m01 copy (both gate pass2 of channel 0)
                cp(False, st_t[:, 0:1, :], pq[:, 0:1, 0:_H])

    nc.finalize()
    _prog_cache[key] = nc
    return nc


def _pack_x(xb, np_e3):
    """xb (64,384,384) f32 -> (64,128,1152) e3m4: [c, p, kc*384+w]."""
    v = xb.reshape(_C, 3, 128, _H).transpose(0, 2, 1, 3).reshape(_C, 128, 3 * _H)
    return np.ascontiguousarray(v.astype(np_e3))


def kernel(x, params, _trace=False):
    from concourse.bass_utils import run_bass_kernel_spmd
    import concourse.mybir as mybir

    x = np.ascontiguousarray(np.asarray(x, dtype=np.float32))
    params = np.asarray(params, dtype=np.float32)
    B = x.shape[0]
    assert x.shape == (_NCORES, _C, _H, _H), x.shape

    k_int = np.trunc(params[:, 0].astype(np.float32))
    k_sel = np.floor(
        np.float32(5.0) + np.float32(5.0) * _sigmoid32(k_int)
    ).astype(np.int32)
    sigma = np.float32(0.5) + np.float32(4.5) * _sigmoid32(params[:, 1])

    np_e3 = mybir.dt.np(mybir.dt.float8e3)

    nc = _build_program()
    in_maps = []
    for b in range(B):
        A = _build_A(int(k_sel[b]), float(sigma[b]))
        at = np.ascontiguousarray(A.T.astype(np.float16))
        in_maps.append({"x8": _pack_x(x[b], np_e3), "at": at})

    res = run_bass_kernel_spmd(nc, in_maps, list(range(_NCORES)), trace=_trace)
    out = np.stack(
        [np.asarray(res.results[b]["out"]).astype(np.float32) for b in range(B)]
    )
    if _trace:
        return out, res
    return out


# revision 9
# speedup vs baseline: 1.1074x; 1.0079x over previous
"""Per-sample Gaussian blur on 8 Trainium2 cores — v3.

Math (as baseline): out_c = A @ X_c @ A^T via two banded tensor-engine
passes; A built on host per sample from (k, sigma).

v3 vs baseline:
- x ships as float8_e3m4 (1 byte), host-prepacked partition-major so DMA
  descriptors are 1152B (full 360GB/s). Mixed-dtype matmul: lhsT=e3
  stationary, rhs=A^T fp16 moving — same 1 cyc/row as fp16. Input DMA
  26us instead of 52us. Measured end-to-end rel_fro error 1.34e-2.
- PSUM repartition: all drains are 2-bank pair copies with >=1-channel
  reuse distance (kills the ps2 single-bank reuse stall of the 2-pool
  layout): ps1p[2]x2 (pass1 m0,m1), psPQ[2]x1 (pass1 m2(c) paired with
  pass2 it0(c-1)), psRS[2]x1 (pass2 it1+it2 of c-1). The PQ pair drains
  into a combined staging tile st(c) = [pt_m2(c) | O_it0..it2(c-1)] so
  one copy serves both passes; output DMA reads st[:, 1:4].
- Copy work balanced across ACT and DVE by assignment tables.
"""

import numpy as np

_H = 384
_C = 64
_NCORES = 8

BATCH_IN = 4         # channels per input DMA instruction
TAIL_ACT = 1         # how many final channels' out-DMAs use the ACT queue

# Engine assignment for the three pair-copies per channel, tuned on the
# cost model: odd channels run m01 on DVE and pq+rs on ACT; even channels
# the mirror image. Alternating the m01 engine decouples the tight
# single-buffered ps1p mm->copy->mm cycle from any one engine's in-order
# queue (96.7us vs 102.6us for fixed assignment).
PQ_ACT = {c for c in range(_C) if c % 2 == 1} | {8, 18, 28, 38, 48, 58}
RS_ACT = {c for c in range(_C) if c % 2 == 1}
M01_DVE = {c for c in range(_C) if c % 2 == 1}
BUFS_T1 = 3
BUFS_ST = 3
PS_BUFS = (1, 1, 2)               # bufs for (ps1p, psPQ, psRS); sum*2 <= 8

_prog_cache = {}


def _sigmoid32(v):
    v = np.asarray(v, dtype=np.float32)
    return (1.0 / (1.0 + np.exp(-v.astype(np.float64)))).astype(np.float32)


def _gauss1d(k, sigma):
    c = np.arange(k, dtype=np.float64) - k // 2
    g = np.exp(-(c * c) / (2.0 * float(sigma) ** 2))
    return g / g.sum()


def _build_A(k, sigma, H=_H):
    """Combined conv(+resize for even k) operator along one axis (H x H)."""
    pad = k // 2
    Ho = H + 2 * pad - k + 1
    g = _gauss1d(k, sigma)
    S = np.zeros((Ho, H), dtype=np.float64)
    for i in range(Ho):
        lo = max(0, i - pad)
        hi = min(H, i - pad + k)
        for m in range(lo, hi):
            S[i, m] = g[m - i + pad]
    if Ho == H:
        return S.astype(np.float32)
    R = np.zeros((H, Ho), dtype=np.float64)
    scale = Ho / H
    for i in range(H):
        src = (i + 0.5) * scale - 0.5
        i0 = int(np.floor(src))
        t = src - i0
        i0c = min(max(i0, 0), Ho - 1)
        i1c = min(max(i0 + 1, 0), Ho - 1)
        R[i, i0c] += 1.0 - t
        R[i, i1c] += t
    return (R @ S).astype(np.float32)


def _build_program():
    key = ("v3", TAIL_ACT, BATCH_IN, tuple(sorted(PQ_ACT)), tuple(sorted(RS_ACT)),
           tuple(sorted(M01_DVE)), PS_BUFS, BUFS_T1, BUFS_ST)
    if key in _prog_cache:
        return _prog_cache[key]

    from contextlib import ExitStack
    import concourse.bacc as bacc
    import concourse.mybir as mybir
    import concourse.tile as tile

    f32 = mybir.dt.float32
    f16 = mybir.dt.float16
    e3 = mybir.dt.float8e3

    nc = bacc.Bacc(None, target_bir_lowering=False)
    # x prepacked on host: x8[c, p, kc*384 + w] = e3m4(X[c, kc*128 + p, w])
    x_d = nc.declare_dram_parameter("x8", [_C, 128, 3 * _H], e3, isOutput=False)
    at_d = nc.declare_dram_parameter("at", [_H, _H], f16, isOutput=False)
    out_d = nc.declare_dram_parameter("out", [_C, _H, _H], f16, isOutput=True)

    # Banded plan: A^T chunk kc has nonzero cols only in [128*kc-4, 128*kc+132)
    MM_PLAN = [(0, 0, 132, True), (1, 124, 260, False), (2, 252, 384, False)]
    n_mm = len(MM_PLAN)

    def cp(act, dst, src):
        if act:
            nc.scalar.copy(dst, src)
        else:
            nc.vector.tensor_copy(dst, src)

    with tile.TileContext(nc) as tc, ExitStack() as ctx:
        at_pool = ctx.enter_context(tc.tile_pool(name="at", bufs=1))
        x_pool = ctx.enter_context(tc.tile_pool(name="x", bufs=4))
        t1_pool = ctx.enter_context(tc.tile_pool(name="t1", bufs=BUFS_T1))
        st_pool = ctx.enter_context(tc.tile_pool(name="st", bufs=BUFS_ST))
        psALL = ctx.enter_context(
            tc.tile_pool(name="psALL", bufs=4, space="PSUM"))
        ps1p = psPQ = psRS = psALL

        at_t = at_pool.tile([128, 3, _H], f16)
        nc.gpsimd.dma_start(
            at_t[:], at_d[:].rearrange("(kc p) i -> p kc i", p=128)
        )

        x_tiles = {}
        pend = {}   # c -> (t1_t, st_t) holding P^T of channel c
        for step in range(_C + 1):
            if step < _C and step % BATCH_IN == 0:
                bi = step // BATCH_IN
                xt = x_pool.tile([128, BATCH_IN, 3, _H], e3, name="xt")
                src = x_d[step : step + BATCH_IN].rearrange(
                    "c p (kc w) -> p c kc w", kc=3
                )
                if bi == 0:
                    for i in range(BATCH_IN):
                        nc.sync.dma_start(xt[:, i], src[:, i])
                else:
                    nc.sync.dma_start(xt[:], src)
                for i in range(BATCH_IN):
                    x_tiles[step + i] = (xt, i)

            c = step            # pass1 channel
            d = step - 1        # pass2 channel
            st_t = st_pool.tile([128, 4, _H], f16, name="stt")
            pq = None
            rs = None

            def mm(dst, lhsT, plan_kc, lo, hi, st_flag, stop_flag):
                nc.tensor.matmul(
                    dst, lhsT, at_t[:, plan_kc, lo:hi],
                    start=st_flag, stop=stop_flag, skip_group_check=True,
                )

            if c < _C:
                xt, xi = x_tiles.pop(c)
                t1_t = t1_pool.tile([128, 2, _H], f16, name="t1t")
                # pass1 m0, m1 -> ps1p pair
                p1p = ps1p.tile([128, 2, 512], f32, name="pspair")
                for m in range(2):
                    for i_mm, (kc, lo, hi, stf) in enumerate(MM_PLAN):
                        mm(p1p[:, m, lo:hi], xt[:, xi, kc, 128 * m : 128 * (m + 1)],
                           kc, lo, hi, stf, i_mm == n_mm - 1)
                # pair-copy m0,m1 — emitted before m2 group
                cp(c not in M01_DVE, t1_t[:], p1p[:, :, 0:_H])
                pq = psALL.tile([128, 2, 512], f32, name="pspair")
                # pass1 m2 -> pq slot 0
                for i_mm, (kc, lo, hi, stf) in enumerate(MM_PLAN):
                    mm(pq[:, 0, lo:hi], xt[:, xi, kc, 256:384],
                       kc, lo, hi, stf, i_mm == n_mm - 1)
                pend[c] = (t1_t, st_t)

            if d >= 0:
                if pq is None:
                    pq = psALL.tile([128, 2, 512], f32, name="pspair")
                rs = psALL.tile([128, 2, 512], f32, name="pspair")
                t1_p, st_p = pend.pop(d)

                def lhs2(kc, it):
                    sl = slice(128 * it, 128 * (it + 1))
                    if kc < 2:
                        return t1_p[:, kc, sl]
                    return st_p[:, 0, sl]

                # it0 -> pq slot 1
                for i_mm, (kc, lo, hi, stf) in enumerate(MM_PLAN):
                    mm(pq[:, 1, lo:hi], lhs2(kc, 0), kc, lo, hi, stf,
                       i_mm == n_mm - 1)
                # pair-copy [pt_m2(c) | O_it0(d)] -> st(c)[:, 0:2]
                if c < _C:
                    cp(c in PQ_ACT, st_t[:, 0:2, :], pq[:, :, 0:_H])
                else:
                    # final step: DVE drains it0(63) while ACT runs rs(63),
                    # so the tail DMA's deps complete in parallel
                    cp(False, st_t[:, 1:2, :], pq[:, 1:2, 0:_H])
                # it1, it2 -> rs
                for it in (1, 2):
                    for i_mm, (kc, lo, hi, stf) in enumerate(MM_PLAN):
                        mm(rs[:, it - 1, lo:hi], lhs2(kc, it), kc, lo, hi, stf,
                           i_mm == n_mm - 1)
                cp(d in RS_ACT, st_t[:, 2:4, :], rs[:, :, 0:_H])
                # output channel d lives in st(c)[:, 1:4]
                odst = out_d[d].rearrange("(m p) j -> p m j", p=128)
                if d >= _C - TAIL_ACT:
                    # tail: ACT's HWDGE queue is idle after its last copy and
                    # generates descriptors ~0.6us faster than Pool SWDGE
                    nc.scalar.dma_start(odst, st_t[:, 1:4, :])
                else:
                    nc.gpsimd.dma_start(odst, st_t[:, 1:4, :])
            elif c < _C:
                # step 0: no pass2; drain pt_m2 on idle DVE so it runs in
                # parallel with ACT's m01 copy (both gate pass2 of channel 0)
                cp(False, st_t[:, 0:1, :], pq[:, 0:1, 0:_H])

    nc.finalize()
    _prog_cache[key] = nc
    return nc


def _pack_x(xb, np_e3):
    """xb (64,384,384) f32 -> (64,128,1152) e3m4: [c, p, kc*384+w]."""
    v = xb.reshape(_C, 3, 128, _H).transpose(0, 2, 1, 3).reshape(_C, 128, 3 * _H)
    return np.ascontiguousarray(v.astype(np_e3))


def kernel(x, params, _trace=False):
    from concourse.bass_utils import run_bass_kernel_spmd
    import concourse.mybir as mybir

    x = np.ascontiguousarray(np.asarray(x, dtype=np.float32))
    params = np.asarray(params, dtype=np.float32)
    B = x.shape[0]
    assert x.shape == (_NCORES, _C, _H, _H), x.shape

    k_int = np.trunc(params[:, 0].astype(np.float32))
    k_sel = np.floor(
        np.float32(5.0) + np.float32(5.0) * _sigmoid32(k_int)
    ).astype(np.int32)
    sigma = np.float32(0.5) + np.float32(4.5) * _sigmoid32(params[:, 1])

    np_e3 = mybir.dt.np(mybir.dt.float8e3)

    nc = _build_program()
    in_maps = []
    for b in range(B):
        A = _build_A(int(k_sel[b]), float(sigma[b]))
        at = np.ascontiguousarray(A.T.astype(np.float16))
        in_maps.append({"x8": _pack_x(x[b], np_e3), "at": at})

    res = run_bass_kernel_spmd(nc, in_maps, list(range(_NCORES)), trace=_trace)
    out = np.stack(
        [np.asarray(res.results[b]["out"]).astype(np.float32) for b in range(B)]
    )
    if _trace:
        return out, res
    return out
